# revision 19
# baseline (speedup 1.0000x reference)
"""ConvLoRA fused kernel for Trainium2 (8 NeuronCores, data-parallel over batch).

Math: conv is linear in its weight, so
    org + outA + outB = conv(x[b], conv_w + wA[b] + wB[b]) + conv_b
One fused per-sample 3x3 conv (256->256ch) instead of three. The per-sample
fused weights come from the tiny MLP + LoRA factor product (0.03% of total
FLOPs) computed on the host and shipped as [cout-half, tap, cin] bf16 tiles.

The conv runs in bf16 (rel err ~2e-3, well under tolerance): 18
PSUM-accumulated matmuls (9 taps x 2 cin-chunks) per (cout-half, 4-row pixel
tile), N=512 free dim, which streams at the PE issue floor (~216 ns/matmul
vs 236 for fp32r). Inputs are host-padded (no edge memsets) and laid out
partition-major so each 34-row slab is ONE dma (dma_start issue costs
~650ns of queue-engine time, so few big DMAs beat many small ones — this
was 23us of the old startup). Input DMAs ride the sync ring, outputs the
scalar ring; the first slab is row-split across both rings so the first
chain unblocks ~3us after the preamble. A burst of dummy matmuls warms the
PE HAM clock gate (cold PE runs at 1.2 GHz for the first ~3.4us) while the
first real DMAs are in flight.
"""
import sys
sys.path.insert(0, '/opt/trn_rl_repo')
import numpy as np
import ml_dtypes

import concourse.bacc as bacc
import concourse.mybir as mybir
import concourse.tile as tile
from concourse.bass_utils import run_bass_kernel_spmd

f32 = mybir.dt.float32
bf16 = mybir.dt.bfloat16

B, CIN, COUT, KS, H, W, R = 16, 256, 256, 3, 128, 128, 16
NCORES = 8
NB = B // NCORES   # 2 samples per core
NSLAB = 4          # row slabs per image
ROWS = H // NSLAB  # 32 output rows per slab
NPT = ROWS // 4    # 4-row pixel tiles per slab
BF = np.dtype(ml_dtypes.bfloat16)


def _build_nc():
    nc = bacc.Bacc("TRN2", target_bir_lowering=False, debug=False, num_devices=NCORES)

    # x host-padded to 130x130 (zero border), partition-major [b, c, j, row, col]
    x_loc = nc.dram_tensor("x_loc", [NB, 128, 2, H + 2, W + 2], bf16, kind="ExternalInput")
    # fused per-sample weights [b, c, oc, t*2+j, m]  (cout = oc*128 + m)
    wk = nc.dram_tensor("wk", [NB, 128, 2, 18, 128], bf16, kind="ExternalInput")
    convb = nc.dram_tensor("convb", [128, 2], f32, kind="ExternalInput")
    out = nc.dram_tensor("out", [NB, 128, 2, H * W], bf16, kind="ExternalOutput")

    with tile.TileContext(nc) as tc:
        from contextlib import ExitStack
        with ExitStack() as ctx:
            cpool = ctx.enter_context(tc.tile_pool(name="consts", bufs=1))
            wpool = ctx.enter_context(tc.tile_pool(name="wpool", bufs=NB))
            xpool = ctx.enter_context(tc.tile_pool(name="xslab", bufs=6))
            stg = ctx.enter_context(tc.tile_pool(name="stg", bufs=12))
            cps = ctx.enter_context(tc.tile_pool(name="cps", bufs=7, space="PSUM"))
            wps = ctx.enter_context(tc.tile_pool(name="wps", bufs=1, space="PSUM"))

            # PE warmup: ~3us of tiny matmuls on a memset tile so the HAM
            # clock-gate opens (1.2 -> 2.4 GHz) before the real work lands.
            # PE warmup: ~3us of tiny matmuls on a memset tile so the HAM
            # clock-gate opens (1.2 -> 2.4 GHz) right as the real work lands.
            # PE warmup: tiny matmuls on a memset tile so the HAM clock-gate
            # opens (1.2 -> 2.4 GHz) right as the real work lands (~11.5us).
            wu = cpool.tile([128, 64], bf16)
            nc.vector.memset(wu[:], 0.0)
            warm_ps = wps.tile([64, 64], f32)
            NWARM = 72
            for i in range(NWARM):
                nc.tensor.matmul(warm_ps[:], wu[:, 0:64], wu[:, 0:64],
                                 start=(i == 0), stop=(i == NWARM - 1))

            # Startup is DMA-delivery-bound (~200GB/s/ring early, ~380 both):
            # split the critical bytes across the two rings — weights on
            # sync, first-slab row pieces on scalar — ordered so the first
            # two chains' inputs transfer before any prefetch. Steady-state
            # outputs ride scalar; prefetch rides sync.
            w_sb = [None] * NB

            def load_w_half(bi, oc, half=None):
                if w_sb[bi] is None:
                    w_sb[bi] = wpool.tile([128, 2, 18, 128], bf16, tag="wk",
                                          name=f"w{bi}")
                a, b = (0, 18) if half is None else half
                nc.sync.dma_start(w_sb[bi][:, oc, a:b], wk[bi, :, oc, a:b])

            slab_tiles = {}

            def load_slab(bi, s, pieces=None, eng=None):
                r0 = s * ROWS
                xx = xpool.tile([128, 2, ROWS + 2, 130], bf16, tag="xslab", name="xx")
                for a, b in (pieces or ((0, ROWS + 2),)):
                    (eng or nc.sync).dma_start(xx[:, :, a:b, :],
                                               x_loc[bi, :, :, r0 + a:r0 + b, :])
                slab_tiles[(bi, s)] = xx
                return xx

            load_w_half(0, 0, half=(0, 9))          # first chain's first 9 weights
            load_slab(0, 0, pieces=((0, 6), (6, 14), (14, 24), (24, 34)),
                      eng=nc.scalar)                # rows for pp0 first
            load_w_half(0, 0, half=(9, 18))
            load_w_half(0, 1)
            convb_sb = cpool.tile([128, 2], f32)
            nc.sync.dma_start(convb_sb[:], convb[:])
            for s in range(1, NSLAB):               # rest of sample 0 on sync
                load_slab(0, s)

            for bi in range(NB):
                for s in range(NSLAB):
                    xt = slab_tiles.get((bi, s)) or load_slab(bi, s)
                    if s == 1 and bi + 1 < NB:
                        load_w_half(bi + 1, 0)
                        load_w_half(bi + 1, 1)
                    last_slab = (bi == NB - 1 and s == NSLAB - 1)
                    for pp in range(NPT):
                        st = stg.tile([128, 2, 4, 128], bf16, tag="stg")
                        y0 = s * ROWS + 4 * pp
                        for oc in range(2):
                            ps = cps.tile([128, 4, 128], f32, tag="cps")
                            k = 0
                            for kh in range(3):
                                for kw in range(3):
                                    t = kh * 3 + kw
                                    for j in range(2):
                                        nc.tensor.matmul(
                                            ps[:],
                                            w_sb[bi][:, oc, t * 2 + j, :],
                                            xt[:, j, 4 * pp + kh:4 * pp + kh + 4, kw:kw + 128],
                                            start=(k == 0), stop=(k == 17))
                                        k += 1
                            nc.vector.tensor_scalar_add(st[:, oc], ps[:], convb_sb[:, oc:oc + 1])
                            if last_slab:
                                # stream per-oc so the final transfer starts
                                # as soon as its chain drains
                                nc.scalar.dma_start(
                                    out[bi, :, oc, y0 * W:(y0 + 4) * W],
                                    st[:, oc].rearrange("p r w -> p (r w)"))
                        if not last_slab:
                            nc.scalar.dma_start(
                                out[bi, :, :, y0 * W:(y0 + 4) * W],
                                st[:].rearrange("p o r w -> p o (r w)"))

    nc.finalize()
    return nc


def _host_prep(inputs):
    """Replicate the reference's weight math on host; pad + shard x."""
    x = np.asarray(inputs["x"], np.float32)
    wms = np.asarray(inputs["wms"], np.float32)
    conv_w = np.asarray(inputs["conv_w"], np.float32)
    conv_b = np.asarray(inputs["conv_b"], np.float32)
    lora_B = np.asarray(inputs["lora_B"], np.float32)

    def g(name):
        return np.asarray(inputs[name], np.float32)

    def embed(v, w1, b1, w2, b2):
        h = v @ w1.T + b1
        h = np.where(h >= 0, h, 0.2 * h)
        return h @ w2.T + b2

    coff1 = embed(wms[0], g("e1_w1"), g("e1_b1"), g("e1_w2"), g("e1_b2")).reshape(B, R, R)
    coff2 = embed(wms[1], g("e2_w1"), g("e2_b1"), g("e2_w2"), g("e2_b2")).reshape(B, R, R)

    def lora_w(coff, lora_A):
        # (COUT*K, R) @ (B,R,R) @ (R, CIN*K) -> (B, COUT, CIN, K, K)
        m = np.einsum('brq,qc->brc', coff, lora_A)
        w = np.einsum('pr,brc->bpc', lora_B, m)
        return w.reshape(B, COUT, CIN, KS, KS)

    wf = conv_w[None] + lora_w(coff1, g("lora_A1")) + lora_w(coff2, g("lora_A2"))
    # [b, kh, kw, cin, cout] -> [b, c, oc, t2j, m] bf16
    wkh = wf.transpose(0, 3, 4, 2, 1).reshape(B, 18, 128, 2, 128)
    wkh = np.ascontiguousarray(wkh.transpose(0, 2, 3, 1, 4)).astype(BF)

    xb = x.astype(BF)  # quantize once; all taps see identical bits
    xpad = np.zeros((B, 128, 2, H + 2, W + 2), BF)
    xpad[:, :, :, 1:H + 1, 1:W + 1] = xb.reshape(B, 2, 128, H, W).transpose(0, 2, 1, 3, 4)

    convb = np.ascontiguousarray(conv_b.reshape(2, 128).T.astype(np.float32))

    in_maps = []
    for core in range(NCORES):
        b0 = core * NB
        in_maps.append({
            "x_loc": np.ascontiguousarray(xpad[b0:b0 + NB]),
            "wk": np.ascontiguousarray(wkh[b0:b0 + NB]),
            "convb": convb,
        })
    return in_maps


_NC = None


def kernel(**inputs) -> np.ndarray:
    global _NC
    if _NC is None:
        _NC = _build_nc()
    in_maps = _host_prep(inputs)
    res = run_bass_kernel_spmd(_NC, in_maps, core_ids=list(range(NCORES)))
    parts = [np.asarray(res.results[c]["out"]).reshape(NB, 128, 2, H, W)
             .transpose(0, 2, 1, 3, 4).reshape(NB, COUT, H, W).astype(np.float32)
             for c in range(NCORES)]
    return np.concatenate(parts, axis=0)
